# revision 31
# baseline (speedup 1.0000x reference)
"""Gated multi-head self-attention on 8 Trainium2 NeuronCores.

Reference computation (per batch b of 4, N=1024 tokens, 8 heads x 64):
    q  = (x @ wq.T) * 64**-0.5
    k,v = split(x @ wkv.T)
    dots = q k^T + bias;  attn = softmax(dots)
    out  = (attn @ v) * sigmoid(x @ wg.T + bg)
    y    = out @ wo.T + bo                # bo added on host after gather

Sharding: token-sharded, zero collectives. Core c handles batch b=c//2 and
query-token half c%2 (512 query rows). Each core computes K/V for its whole
batch.

Schedule notes:
  - Q/K projections run in fp8e4 DoubleRow mode; weights pre-scaled by 64 on
    the host, descale folded into the Exp activation's `scale` argument.
  - Softmax is unnormalized exp multiplied by exp(bias^T) (host fp16); the
    denominators come free from 64 ones-columns appended to V in the AV
    matmul.
  - Gating uses a combined denominator: gated = av / (denom * (1 + e^-g)),
    with a single-pass reciprocal_approx_fast on the DVE. The scalar engine
    only ever runs Exp (+Copy), so its activation table loads exactly once.
  - The scalar Exp stream is the metronome (32 x ~0.93us). Everything else is
    arranged around keeping it dense: only kT/qT-ct0 copies precede it, the
    4 sigmoid-exps slot in after exps(hp1) when their inputs are long ready.
  - DMA priority: sync queue carries x8/wk8/wq8 then the 16 exp-bias chunks;
    vector queue carries bgn/xbT/wgT/wvT/woT in consumption order.
  - PSUM: phase1 proj(4)+dots(4) banks; phase2 dots(4)+av(4); tail y(4).
"""

import sys

if "/opt/trn_rl_repo" not in sys.path:
    sys.path.insert(0, "/opt/trn_rl_repo")

import ml_dtypes
import numpy as np

import concourse.bass as bass  # noqa: F401  (AP helpers)
import concourse.mybir as mybir
import concourse.tile as tile
from concourse import bacc
from concourse.bass_utils import run_bass_kernel_spmd

F32 = mybir.dt.float32
F16 = mybir.dt.float16
F8 = mybir.dt.float8e4
AF = mybir.ActivationFunctionType
ALU = mybir.AluOpType
DR = mybir.MatmulPerfMode.DoubleRow
FP8_NP = ml_dtypes.float8_e4m3

P = 128
HEADS = 8
DH = 64
DIM = 512
N = 1024  # tokens per batch (kv length)
NQ = 512  # query tokens per core
B = 4
N_CORES = 8
DT = DIM // P  # 4 channel tiles
JT = N // P  # 8 kv-token tiles
HP = HEADS // 2  # 4 head pairs

W_SCALE = 64.0  # host-side fp8 weight scale for wq/wk
EXP_SCALE = 1.0 / (W_SCALE * W_SCALE * 8.0)  # descale + dim_head**-0.5

N_WARM = 48
GP_MULT_JTS = (3, 6)  # eb-mult tiles offloaded to gpsimd per head pair


def build_nc():
    nc = bacc.Bacc(None, target_bir_lowering=False, debug=False)

    # Per-core inputs. Token order inside x is "query half first".
    # f8p packs x8 | wk8 | wq8 | wg8 column-wise: one DMA with ~2.5KB
    # descriptors (separate fp8 tensors would use 512B descriptors and get
    # starved in the DMA engines' round-robin against the f16 queue)
    F8W = N + 3 * DIM
    f8p_d = nc.dram_tensor("f8p", [DIM, F8W], F8, kind="ExternalInput")
    xbT_d = nc.dram_tensor("xbT", [DIM, N], F16, kind="ExternalInput")
    wvT_d = nc.dram_tensor("wvT", [DIM, DIM], F16, kind="ExternalInput")
    woT_d = nc.dram_tensor("woT", [DIM, DIM], F16, kind="ExternalInput")
    bgn_d = nc.dram_tensor("bgn", [DIM], F32, kind="ExternalInput")  # -bg
    bT_d = nc.dram_tensor("bT", [HP, N // 256, P, 2, 2, NQ], F16, kind="ExternalInput")
    y_d = nc.dram_tensor("y", [NQ, DIM], F16, kind="ExternalOutput")

    with tile.TileContext(nc) as tc:
        with (
            tc.tile_pool(name="const", bufs=1) as const,
            tc.tile_pool(name="work", bufs=1) as work,
            tc.tile_pool(name="attn", bufs=20) as attn_pool,
            tc.tile_pool(name="rec", bufs=2) as rec_pool,
            tc.tile_pool(name="ebuf", bufs=6) as ebuf,
            tc.tile_pool(name="yout", bufs=4) as yout,
        ):
            # ---- constants; DMA queues by priority -----------------------
            warm_sb = const.tile([P, P], F16, tag="warm", name="warm")
            nc.vector.memset(warm_sb[:], 1.0)

            # sync queue: the packed fp8 operands, then exp-bias chunks
            f8p = const.tile([P, DT, F8W], F8, tag="f8p", name="f8p")
            nc.sync.dma_start(f8p[:], f8p_d.rearrange("(o p) m -> p o m", p=P))
            x8 = f8p[:, :, 0:N]
            wk8 = f8p[:, :, N : N + DIM]
            wq8 = f8p[:, :, N + DIM : N + 2 * DIM]
            wg8 = f8p[:, :, N + 2 * DIM : N + 3 * DIM]

            # gpsimd queue: gating bias, f16 operands in consumption order,
            # then the hp2/hp3 exp-bias chunks pre-dispatched (their SBUF
            # tiles are dedicated, so the transfers pace themselves), then woT
            bgn_sb = const.tile([P, DT], F32, tag="bgn", name="bgn")
            nc.gpsimd.dma_start(bgn_sb[:], bgn_d.rearrange("(o p) -> p o", p=P))
            xbT = const.tile([P, DT, N], F16, tag="xbT", name="xbT")
            wvT = const.tile([P, DT, DIM], F16, tag="wvT", name="wvT")

            def emit_mid_dmas():
                # xbT/wvT are not needed until the v projections (~10us
                # after the fp8 pack); dispatching them mid-schedule gives
                # the pack the DMA engines to itself at startup
                nc.gpsimd.dma_start(xbT[:], xbT_d.rearrange("(o p) m -> p o m", p=P))
                nc.gpsimd.dma_start(wvT[:], wvT_d.rearrange("(o p) m -> p o m", p=P))

            # hp2/hp3 exp-bias tiles + woT: dispatched mid-schedule (see
            # emit_late_dmas) so their 4.5MB doesn't compete with the
            # critical startup loads
            eb_hi = {}
            for hp in (2, 3):
                for pr in range(4):
                    eb_hi[(hp, pr)] = const.tile(
                        [P, 2, 2 * NQ], F16, tag=f"ebh{hp}{pr}", name="ebh"
                    )
            woT = const.tile([P, DT, DIM], F16, tag="woT", name="woT")

            def emit_late_dmas():
                for hp in (2, 3):
                    for pr in range(4):
                        nc.gpsimd.dma_start(eb_hi[(hp, pr)][:], bT_d[hp, pr])
                nc.gpsimd.dma_start(
                    woT[:], woT_d.rearrange("(o p) m -> p o m", p=P)
                )

            # persistent activations
            kT = [work.tile([P, N], F16, tag=f"kT{t}", name=f"kT{t}") for t in range(DT)]
            qT = [work.tile([P, NQ], F16, tag=f"qT{t}", name=f"qT{t}") for t in range(DT)]
            v_aug = [work.tile([P, HEADS * P], F16, tag=f"vaug{j}", name=f"vaug{j}") for j in range(JT)]
            egT = [work.tile([P, NQ], F16, tag=f"eg{t}", name=f"eg{t}") for t in range(DT)]
            gatedT = [work.tile([P, NQ], F16, tag=f"gated{t}", name=f"gated{t}") for t in range(DT)]

            # ones columns for the softmax denominator (gpsimd: SBUF only)
            for jt in range(JT):
                nc.gpsimd.memset(
                    v_aug[jt].rearrange("p (h c) -> p h c", c=P)[:, :, DH:P], 1.0
                )

            eb_cache = {}
            tiles_by_hp = {}

            def emit_dots_tile(hp, jt):
                ct = hp
                if hp < 2:
                    if jt % 2 == 0:
                        eb2 = ebuf.tile([P, 2, 2 * NQ], F16, tag="eb", name="eb")
                        nc.sync.dma_start(eb2[:], bT_d[hp, jt // 2])
                        eb_cache[0] = eb2
                    eb = eb_cache[0][:, jt % 2, :]
                else:
                    eb = eb_hi[(hp, jt // 2)][:, jt % 2, :]
                dps = ps_dots.tile([P, 2 * NQ], F32, tag="dots", name="dots")
                for s in range(2):
                    lo = s * DH
                    nc.tensor.matmul(
                        dps[:, s * NQ : (s + 1) * NQ],
                        kT[ct][lo : lo + DH, jt * P : (jt + 1) * P],
                        qT[ct][lo : lo + DH, :],
                        start=True,
                        stop=True,
                        tile_position=(lo, 0),
                    )
                at = attn_pool.tile([P, 2 * NQ], F16, tag="attn", name="attn")
                nc.scalar.activation(out=at[:], in_=dps[:], func=AF.Exp, scale=EXP_SCALE)
                meng = nc.gpsimd if (jt in GP_MULT_JTS and hp < HP - 1) else nc.vector
                meng.tensor_tensor(at[:], at[:], eb[:], ALU.mult)
                tiles_by_hp.setdefault(hp, []).append(at)
                return at

            def emit_kq_proj(w8, dst, ct, jc, nq, cast_eng):
                ps = ps_proj.tile([P, NQ], F32, tag="proj", name="proj")
                for kp in (0, 2):
                    nc.tensor.matmul(
                        ps[:],
                        w8[:, kp : kp + 2, ct * P : (ct + 1) * P],
                        x8[:, kp : kp + 2, jc * NQ : jc * NQ + nq],
                        start=(kp == 0),
                        stop=(kp == 2),
                        perf_mode=DR,
                    )
                if cast_eng is nc.scalar:
                    nc.scalar.activation(
                        out=dst[:, jc * NQ : (jc + 1) * NQ], in_=ps[:], func=AF.Copy
                    )
                else:
                    cast_eng.tensor_copy(out=dst[:, jc * NQ : (jc + 1) * NQ], in_=ps[:])

            def emit_gates(ct):
                ps = ps_proj.tile([P, NQ], F32, tag="proj", name="proj")
                for kp in (0, 2):
                    nc.tensor.matmul(
                        ps[:],
                        wg8[:, kp : kp + 2, ct * P : (ct + 1) * P],
                        x8[:, kp : kp + 2, 0:NQ],
                        start=(kp == 0),
                        stop=(kp == 2),
                        perf_mode=DR,
                    )
                return ps  # scalar exp emitted separately (emit_eg)

            def emit_eg(ct, ps):
                # eg = exp(-(g + bg)); gates are fp8 with weights scaled by
                # 64, so the exp descale is -1/64. Reuses the Exp table.
                nc.scalar.activation(
                    out=egT[ct][:], in_=ps[:], func=AF.Exp, scale=-1.0 / W_SCALE,
                    bias=bgn_sb[:, ct : ct + 1],
                )
                # egp1 = 1 + eg, in place (f16, 2x DVE mode)
                nc.vector.tensor_scalar_add(egT[ct][:], egT[ct][:], 1.0)

            def emit_v(jt):
                ps = ps_proj.tile([P, NQ], F32, tag="proj", name="proj")
                for kt in range(DT):
                    nc.tensor.matmul(
                        ps[:],
                        xbT[:, kt, jt * P : (jt + 1) * P],
                        wvT[:, kt, :],
                        start=(kt == 0),
                        stop=(kt == DT - 1),
                    )
                nc.vector.tensor_copy(
                    out=v_aug[jt].rearrange("p (h c) -> p h c", c=P)[:, :, 0:DH],
                    in_=ps[:].rearrange("p (h c) -> p h c", c=DH),
                )

            def emit_av_jt(av, hp, jt):
                tiles = tiles_by_hp[hp]
                for s in range(2):
                    h = 2 * hp + s
                    nc.tensor.matmul(
                        av[:, s * NQ : (s + 1) * NQ],
                        v_aug[jt][:, h * P : (h + 1) * P],
                        tiles[jt][:, s * NQ : (s + 1) * NQ],
                        start=(jt == 0),
                        stop=(jt == JT - 1),
                    )

            def emit_gating(av, hp):
                # gated = av[v] / (denom * (1 + e^-g)) ; one fast reciprocal
                ct = hp
                cden = rec_pool.tile([P, NQ], F32, tag="cden", name="cden")
                crec = rec_pool.tile([P, NQ], F32, tag="crec", name="crec")
                for s in range(2):
                    lo = s * DH
                    nc.vector.tensor_tensor(
                        cden[lo : lo + DH, :],
                        av[DH:P, s * NQ : (s + 1) * NQ],
                        egT[ct][lo : lo + DH, :],
                        ALU.mult,
                    )
                nc.vector.reciprocal_approx_fast(out=crec[:], in_=cden[:])
                for s in range(2):
                    lo = s * DH
                    nc.vector.tensor_tensor(
                        gatedT[ct][lo : lo + DH, :],
                        av[0:DH, s * NQ : (s + 1) * NQ],
                        crec[lo : lo + DH, :],
                        ALU.mult,
                    )

            # av-step iterator state: one AV accumulation step = both heads
            # of one kv tile; gating is emitted right after a pair completes
            av_state = {"a": 0, "tiles": {}}

            def emit_av_step(ps_av):
                a = av_state["a"]
                hp, jt = divmod(a, JT)
                if jt == 0:
                    av_state["tiles"][hp] = ps_av.tile(
                        [P, 2 * NQ], F32, tag="av", name="av"
                    )
                av = av_state["tiles"][hp]
                emit_av_jt(av, hp, jt)
                if jt == JT - 1:
                    emit_gating(av, hp)
                av_state["a"] = a + 1

            # PSUM is a two-sided stack: dots lives on the right and closes
            # mid-stream (its banks become the y accumulators); proj -> av
            # run sequentially on the left. Manual pool lifetimes keep both
            # sides at <= 4 banks, 8 total.
            ctx_dots = tc.tile_pool(name="ps_dots", bufs=2, space="PSUM", side="right")
            ps_dots = ctx_dots.__enter__()
            if True:
                if True:
                    # ---- phase 1: projections + dots(hp0, hp1) -----------
                    with tc.tile_pool(
                        name="ps_proj", bufs=4, space="PSUM", side="left"
                    ) as ps_proj:
                        # dummy 1x1 exp: pulls the scalar engine's
                        # ACT_TABLE_LOAD into the DMA wait at startup
                        nc.scalar.activation(
                            out=egT[0][0:1, 0:1], in_=warm_sb[0:1, 0:1],
                            func=AF.Exp,
                        )
                        warm_ps = ps_proj.tile([P, NQ], F32, tag="proj", name="proj")
                        for _ in range(N_WARM):
                            nc.tensor.matmul(
                                warm_ps[:, 0:P], warm_sb[:], warm_sb[:],
                                start=True, stop=True,
                            )
                        nc.scalar.activation(
                            out=warm_sb[0:1, 0:1], in_=warm_ps[0:1, 0:1],
                            func=AF.Copy,
                        )

                        # qT0 first (its cast heads the vector queue), all
                        # casts on vector: the scalar queue holds nothing but
                        # the exp stream. All hp0 dots only need ct0; the
                        # ps_dots WAR ring paces the PE to the exp stream and
                        # fillers slot into the slack.
                        emit_kq_proj(wq8, qT[0], 0, 0, NQ, nc.vector)
                        emit_kq_proj(wk8, kT[0], 0, 0, NQ, nc.vector)
                        emit_kq_proj(wk8, kT[0], 0, 1, NQ, nc.vector)
                        emit_dots_tile(0, 0)
                        emit_dots_tile(0, 1)
                        emit_dots_tile(0, 2)
                        emit_mid_dmas()
                        for ct in range(1, DT):
                            emit_kq_proj(wk8, kT[ct], ct, 0, NQ, nc.vector)
                            emit_kq_proj(wk8, kT[ct], ct, 1, NQ, nc.vector)
                            emit_kq_proj(wq8, qT[ct], ct, 0, NQ, nc.vector)
                            emit_dots_tile(0, 2 + ct)
                        emit_eg(0, emit_gates(0))
                        emit_dots_tile(0, 6)
                        emit_eg(1, emit_gates(1))
                        emit_dots_tile(0, 7)
                        emit_eg(2, emit_gates(2))
                        emit_dots_tile(1, 0)
                        emit_eg(3, emit_gates(3))
                        emit_dots_tile(1, 1)
                        emit_v(0)
                        emit_dots_tile(1, 2)
                        emit_v(1)
                        emit_late_dmas()
                        emit_dots_tile(1, 3)
                        emit_v(2)
                        emit_dots_tile(1, 4)
                        emit_v(3)
                        emit_dots_tile(1, 5)
                        emit_v(4)
                        emit_dots_tile(1, 6)
                        emit_v(5)
                        emit_dots_tile(1, 7)
                        emit_v(6)
                        emit_v(7)

                    # ---- phase 2: zip remaining dots with AV steps -------
                    # [dots, av, av]: the AV stream (32 steps, starting 16
                    # behind) catches up to the exps by the end, so the tail
                    # after the last exp is just the final gating chain.
                    ctx_av = tc.tile_pool(
                        name="ps_av", bufs=2, space="PSUM", side="left"
                    )
                    ps_av = ctx_av.__enter__()
                    d_emitted = 16
                    for hp in (2, 3):
                        for jt in range(JT):
                            for _ in range(2):
                                if av_state["a"] <= d_emitted - 5:
                                    emit_av_step(ps_av)
                            emit_dots_tile(hp, jt)
                            d_emitted += 1

            # dots pool closes here; its right-side banks become the y tiles
            ctx_dots.__exit__(None, None, None)
            ctx_y = tc.tile_pool(name="ps_y", bufs=1, space="PSUM", side="right")
            ps_y = ctx_y.__enter__()

            # ---- phase 3: output projection tail -------------------------
            # ct0..2 interleave with the remaining AV steps: the PE fills
            # its exp-wait slack with output-projection work
            ys = [ps_y.tile([P, DIM], F32, tag=f"y{it}", name="y") for it in range(4)]
            for ct in range(DT - 1):
                for it in range(NQ // P):
                    nc.tensor.matmul(
                        ys[it][:],
                        gatedT[ct][:, it * P : (it + 1) * P],
                        woT[:, ct, :],
                        start=(ct == 0),
                        stop=False,
                    )
                for _ in range(2):
                    if av_state["a"] < HP * JT:
                        emit_av_step(ps_av)
            while av_state["a"] < HP * JT:
                emit_av_step(ps_av)
            # ct3 split into two 64-row halves (tile_position pairs) so the
            # it-tiles can start as soon as gated3's s0 half lands; copies
            # chase the s1 halves per tile
            ysb = yout.tile([P, NQ // P, DIM], F16, tag="ysb", name="ysb")
            ydst = y_d.rearrange("(f p) m -> p f m", p=P)
            for it in range(NQ // P):
                nc.tensor.matmul(
                    ys[it][:],
                    gatedT[DT - 1][0:DH, it * P : (it + 1) * P],
                    woT[0:DH, DT - 1, :],
                    start=False,
                    stop=False,
                    tile_position=(0, 0),
                )
            for it in range(NQ // P):
                nc.tensor.matmul(
                    ys[it][:],
                    gatedT[DT - 1][DH:P, it * P : (it + 1) * P],
                    woT[DH:P, DT - 1, :],
                    start=False,
                    stop=True,
                    tile_position=(DH, 0),
                )
                if it % 2 == 0:
                    nc.scalar.activation(out=ysb[:, it, :], in_=ys[it][:], func=AF.Copy)
                else:
                    nc.vector.tensor_copy(out=ysb[:, it, :], in_=ys[it][:])
            nc.scalar.dma_start(ydst[:, 0:2, :], ysb[:, 0:2, :])
            nc.sync.dma_start(ydst[:, 2:4, :], ysb[:, 2:4, :])
            ctx_y.__exit__(None, None, None)
            ctx_av.__exit__(None, None, None)

    nc.compile()
    return nc


_CACHE = {}


def get_nc():
    if "nc" not in _CACHE:
        _CACHE["nc"] = build_nc()
    return _CACHE["nc"]


def make_in_maps(x, attn_bias, wq, wkv, wo, wg, bg):
    """Host-side sharding: per-core input dicts (weights shared by reference)."""
    x = np.asarray(x, np.float32)
    attn_bias = np.asarray(attn_bias, np.float32)
    wqT = np.asarray(wq, np.float32).T
    wkvT = np.asarray(wkv, np.float32).T
    wq8 = (wqT * W_SCALE).astype(FP8_NP)
    wk8 = (wkvT[:, :DIM] * W_SCALE).astype(FP8_NP)
    wvT = np.ascontiguousarray(wkvT[:, DIM:], np.float16)
    wg8 = (np.asarray(wg, np.float32).T * W_SCALE).astype(FP8_NP)
    woT = np.ascontiguousarray(np.asarray(wo, np.float32).T, np.float16)
    bgn = -np.asarray(bg, np.float32)

    ab = np.exp(attn_bias[0])  # [H, N(i), N(j)]
    # bT[r0][h, j, i] = exp(bias)[h, i, j] with j permuted "query half first"
    bT = {}
    for r0 in (0, NQ):
        perm = np.r_[r0 : r0 + NQ, (NQ - r0) : (NQ - r0) + NQ]
        t = ab[:, r0 : r0 + NQ, :].transpose(0, 2, 1)[:, perm, :]
        t = t.reshape(4, 2, 4, 2, 128, NQ).transpose(0, 2, 4, 3, 1, 5)
        bT[r0] = np.ascontiguousarray(t, dtype=np.float16)

    in_maps = []
    for c in range(N_CORES):
        b, r0 = c // 2, (c % 2) * NQ
        perm = np.r_[r0 : r0 + NQ, (NQ - r0) : (NQ - r0) + NQ]
        xperm = x[b][perm].T
        f8p = np.concatenate(
            [xperm.astype(FP8_NP), wk8, wq8, wg8], axis=1
        )
        in_maps.append(
            {
                "f8p": np.ascontiguousarray(f8p),
                "xbT": np.ascontiguousarray(xperm, np.float16),
                "wvT": wvT,
                "woT": woT,
                "bgn": bgn,
                "bT": bT[r0],
            }
        )
    return in_maps


def kernel(x, mask, attn_bias, wq, wkv, wo, bo, wg, bg, **_):
    # mask is all-ones per the problem spec; ignored.
    nc = get_nc()
    in_maps = make_in_maps(x, attn_bias, wq, wkv, wo, wg, bg)
    res = run_bass_kernel_spmd(nc, in_maps, list(range(N_CORES))).results
    y = np.empty((B, N, DIM), np.float32)
    for c in range(N_CORES):
        b, r0 = c // 2, (c % 2) * NQ
        y[b, r0 : r0 + NQ] = res[c]["y"].astype(np.float32)
    y += np.asarray(bo, np.float32)
    return y
